# revision 29
# baseline (speedup 1.0000x reference)
"""Causal multi-head decoder attention on 8 Trainium2 NeuronCores.

Problem shapes (hardcoded): x [B=2, S=2048, D=1024], 16 heads x d_head=64.
Sharding: core c -> (batch b = c//4, head-group hg = c%4 covering 4 heads).
Attention is fully head-local; each core computes the partial output
projection for its 4 heads, and the host sums the 4 partials per batch
(the "output projection all-reduce") during unshard.

On-device layout strategy (per core):
  - host provides xT = x[b].T  [1024, 2048] so Q/K projections directly
    produce qT/kT [64, S] (head-dim on partitions) with no transposes.
  - V is computed in [S, 64] orientation (x-chunk stationary) and stored
    interleaved with a ones-column per head: vaug [128, 16sc, 4h*65].
    The ones-column makes the attn@V matmul also produce the softmax
    denominator row (zaugT [65, 512] = 64 z rows + 1 denom row).
  - scoresT [ki, qi] = kT-chunk.T @ qT-tile (contraction over d_head=64).
    exp() on the scalar engine (batched over 2 PSUM banks per call);
    causal mask applied additively (-1e5) only on diagonal chunks,
    sliced from one host-built [128, 896] mask tile.
  - 1/sqrt(d_head) folded into the K weights host-side.
  - normalize z by broadcasting reciprocal(denom) across partitions
    (K=1 matmul against a ones row) and multiplying on the vector engine.
  - output projection: O stacked per head-pair so contraction is 128-wide.
"""

import os as _os

import numpy as np

import concourse.bass as bass
import concourse.tile as tile
from concourse import mybir
from concourse.bass_utils import run_bass_kernel_spmd

F32 = mybir.dt.float32
F16 = mybir.dt.float16

B, S, D, NH, DH = 2, 2048, 1024, 16, 64
HL = 4            # heads per core
DC = D // 128     # 8 d-chunks
NQT = S // 512    # 4 qi tiles
NSC = S // 128    # 16 128-token chunks
IGNORE = -100000.0

STAGE = int(_os.environ.get("KERNEL_STAGE", "4"))  # 1=proj 2=+attn 3=+norm 4=full

# ---------------------------------------------------------------------------
# Workaround for this walrus build's per-instruction sync-wait budget of one
# ("Too many sync wait commands"): after Tile scheduling, move excess waits
# from any instruction onto same-engine NoOps inserted just before it.
MAX_WAITS = 1


def _split_sync_waits(nc, max_waits=MAX_WAITS):
    k = 0
    for fn in nc.m.functions:
        for bb in fn.blocks:
            insts = bb.instructions
            i = 0
            while i < len(insts):
                ins = insts[i]
                si = ins.sync_info
                if si is not None and len(si.on_wait) > max_waits:
                    waits = list(si.on_wait)
                    extra, keep = waits[:-max_waits], waits[-max_waits:]
                    for j in range(0, len(extra), max_waits):
                        nop = mybir.InstNoOp(
                            name=nc.get_next_instruction_name(), ins=[], outs=[])
                        k += 1
                        nop.engine = ins.engine
                        nop.sync_info = mybir.SyncInfo(
                            on_wait=extra[j:j + max_waits], on_update=[])
                        nc.register_instruction(nop, overwrite=True)
                        insts.insert(i, nop)
                        i += 1
                    ins.sync_info = mybir.SyncInfo(
                        on_wait=keep, on_update=list(si.on_update))
                i += 1
    return k


# ---------------------------------------------------------------------------
def _emit(nc, tc, d):
    xT_d, wqk_d, qkb_d, wv_d, vb_d, ostk_d, mask_d, outT_d = d

    with tc.tile_pool(name="persist", bufs=1) as persist:
        xT = persist.tile([128, DC, S], F16)
        wqk = persist.tile([128, DC, 512], F16)
        wv = persist.tile([128, DC, 256], F16)
        qkb = persist.tile([128, 4], F32)
        vb = persist.tile([128, 260], F16)
        ostk = persist.tile([128, 2, DC, 128], F16)
        maskt = persist.tile([128, 896], F16)
        qkT = persist.tile([128, 4, S], F16)
        vaug = persist.tile([128, NSC, HL * 65], F16)
        zstk = persist.tile([128, 2, S], F16)
        qdup = persist.tile([128, HL, S], F16)
        kdup = persist.tile([128, HL, S], F16)
        ones64 = persist.tile([128, 64], F32)
        nc.vector.memset(ones64[:, :], 1.0)

        nc.sync.dma_start(out=wqk[:, :, :], in_=wqk_d.rearrange("(c p) n -> p c n", p=128))
        nc.sync.dma_start(out=wv[:, :, :], in_=wv_d.rearrange("(c p) n -> p c n", p=128))
        nc.sync.dma_start(out=qkb[:, :], in_=qkb_d[:, :])
        nc.sync.dma_start(out=vb[:, :], in_=vb_d[:, :])
        nc.sync.dma_start(out=maskt[:, :], in_=mask_d[:, :])
        for dc in range(DC):
            for hh in range(2):
                eng = nc.sync if (dc + hh) % 2 == 0 else nc.scalar
                eng.dma_start(
                    out=xT[:, dc, hh * 1024:(hh + 1) * 1024],
                    in_=xT_d[dc * 128:(dc + 1) * 128, hh * 1024:(hh + 1) * 1024])
        nc.sync.dma_start(out=ostk[:, :, :, :], in_=ostk_d.rearrange("p d r c -> r p d c"))

        with (
            tc.tile_pool(name="psP", bufs=2, space="PSUM") as psP,
            tc.tile_pool(name="psS", bufs=2, space="PSUM") as psS,
            tc.tile_pool(name="psZ", bufs=2, space="PSUM") as psZ,
            tc.tile_pool(name="att", bufs=8) as attp,
            tc.tile_pool(name="nrm", bufs=3) as nrm,
            tc.tile_pool(name="ost", bufs=2) as ostp,
        ):
            for st in range(NQT):
                stw = slice(st * 512, (st + 1) * 512)
                # ---- Q/K projections for this s-tile ----
                for r in range(4):
                    ps = psP.tile([128, 512], F32, tag="proj", name=f"qk_{st}_{r}")
                    for dc in range(DC):
                        nc.tensor.matmul(
                            ps,
                            lhsT=wqk[:, dc, r * 128:(r + 1) * 128],
                            rhs=xT[:, dc, stw],
                            start=(dc == 0), stop=(dc == DC - 1),
                        )
                    nc.vector.tensor_scalar_add(
                        out=qkT[:, r, stw], in0=ps, scalar1=qkb[:, r:r + 1])
                # ---- V projections for this s-tile's four 128-chunks ----
                for sc in range(4 * st, 4 * st + 4):
                    ps = psP.tile([128, 256], F32, tag="proj", name=f"v_{sc}")
                    for dc in range(DC):
                        nc.tensor.matmul(
                            ps,
                            lhsT=xT[:, dc, sc * 128:(sc + 1) * 128],
                            rhs=wv[:, dc, :],
                            start=(dc == 0), stop=(dc == DC - 1),
                        )
                    vsl = vaug[:, sc, :].rearrange("p (h c) -> p h c", c=65)
                    nc.vector.tensor_copy(vsl[:, :, 0:64],
                                          ps.rearrange("p (h c) -> p h c", c=64))
                    nc.vector.memset(vsl[:, :, 64:65], 1.0)
                    nc.vector.tensor_add(out=vaug[:, sc, :], in0=vaug[:, sc, :], in1=vb)
                # ---- duplicate the new qT/kT slices across partition halves
                # so a ki-chunk pair runs as two concurrent row-group matmuls
                for h in range(HL):
                    qrt, rt, pb = h // 2, 2 + h // 2, (h % 2) * 64
                    nc.sync.dma_start(out=qdup[0:64, h, stw], in_=qkT[pb:pb + 64, qrt, stw])
                    nc.sync.dma_start(out=qdup[64:128, h, stw], in_=qkT[pb:pb + 64, qrt, stw])
                    nc.sync.dma_start(out=kdup[0:64, h, stw], in_=qkT[pb:pb + 64, rt, stw])
                    nc.sync.dma_start(out=kdup[64:128, h, stw], in_=qkT[pb:pb + 64, rt, stw])

                # ---- attention for qi-tile st (2 heads in flight) ----
                qt = st
                nkc = 4 * (qt + 1)
                for hp in range(2):
                    heads2 = (2 * hp, 2 * hp + 1)
                    zaugs = {}
                    for h in heads2:
                        zaugs[h] = psZ.tile([65, 512], F32, tag="zaug", name=f"zaug_{h}")
                    for ip in range(nkc // 2):
                        ats = {}
                        for h in heads2:
                            sc2 = psS.tile([128, 1024], F32, tag="sc")
                            for half in range(2):
                                kc = 2 * ip + half
                                lo = half * 64
                                nc.tensor.matmul(
                                    sc2[:, half * 512:(half + 1) * 512],
                                    lhsT=kdup[lo:lo + 64, h, kc * 128:(kc + 1) * 128],
                                    rhs=qdup[lo:lo + 64, h, qt * 512:(qt + 1) * 512],
                                    start=True, stop=True,
                                    tile_position=(lo, 0),
                                )
                            at = attp.tile([128, 1024], F16, tag="at")
                            nc.scalar.activation(out=at, in_=sc2,
                                                 func=mybir.ActivationFunctionType.Exp)
                            for half in range(2):
                                kc = 2 * ip + half
                                j = kc - 4 * qt
                                if 0 <= j < 4:  # diagonal chunk: causal mask
                                    nc.vector.tensor_mul(
                                        out=at[:, half * 512:(half + 1) * 512],
                                        in0=at[:, half * 512:(half + 1) * 512],
                                        in1=maskt[:, 384 - 128 * j: 896 - 128 * j],
                                    )
                            ats[h] = at
                        for h in heads2:
                            for half in range(2):
                                kc = 2 * ip + half
                                nc.tensor.matmul(
                                    zaugs[h],
                                    lhsT=vaug[:, kc, 65 * h: 65 * h + 65],
                                    rhs=ats[h][:, half * 512:(half + 1) * 512],
                                    start=(kc == 0), stop=(kc == nkc - 1),
                                )
                    for h in heads2:
                        zaug = zaugs[h]
                        # normalize: z * exp(-ln(denom)); denom broadcast across
                        # partitions via a K=1 matmul against a ones row.
                        rd = nrm.tile([128, 1024], F32, tag="rd")
                        nc.scalar.activation(out=rd[64:65, 0:512], in_=zaug[64:65, :],
                                             func=mybir.ActivationFunctionType.Ln)
                        nc.scalar.activation(out=rd[64:65, 512:1024],
                                             in_=rd[64:65, 0:512],
                                             func=mybir.ActivationFunctionType.Exp,
                                             scale=-1.0)
                        rb = psS.tile([64, 512], F32, tag="sc")
                        nc.tensor.matmul(rb, lhsT=ones64[64:65, :],
                                         rhs=rd[64:65, 512:1024],
                                         start=True, stop=True)
                        rdb = nrm.tile([64, 512], F32, tag="rdb")
                        nc.vector.tensor_copy(rdb[:, :], rb)
                        pair = h // 2
                        if h % 2 == 0:
                            nc.vector.tensor_mul(out=zstk[0:64, pair, stw],
                                                 in0=zaug[0:64, :], in1=rdb[:, :])
                        else:
                            zs = nrm.tile([64, 512], F16, tag="zs")
                            nc.vector.tensor_mul(out=zs[:, :], in0=zaug[0:64, :],
                                                 in1=rdb[:, :])
                            nc.sync.dma_start(out=zstk[64:128, pair, stw], in_=zs[:, :])
            # ---- output projections, deferred to fill the qt=3 tail ----
            for st in range(NQT):
                stw = slice(st * 512, (st + 1) * 512)
                for dc in range(DC):
                    po = psZ.tile([128, 512], F32, tag="zaug", name=f"po_{st}_{dc}")
                    for pair in range(2):
                        nc.tensor.matmul(
                            po,
                            lhsT=ostk[:, pair, dc, :],
                            rhs=zstk[:, pair, stw],
                            start=(pair == 0), stop=(pair == 1),
                        )
                    og = ostp.tile([128, 512], F16, tag="og")
                    nc.vector.tensor_copy(og[:, :], po)
                    nc.sync.dma_start(
                        out=outT_d[dc * 128:(dc + 1) * 128, stw],
                        in_=og[:, :],
                    )


def build_nc() -> bass.Bass:
    nc = bass.Bass()
    xT_d = nc.dram_tensor("xT", [D, S], F16, kind="ExternalInput")
    wqk_d = nc.dram_tensor("wqk", [D, 512], F16, kind="ExternalInput")
    qkb_d = nc.dram_tensor("qkb", [128, 4], F32, kind="ExternalInput")
    wv_d = nc.dram_tensor("wv", [D, 256], F16, kind="ExternalInput")
    vb_d = nc.dram_tensor("vb", [128, 260], F16, kind="ExternalInput")
    ostk_d = nc.dram_tensor("ostk", [2, DC, 128, 128], F16, kind="ExternalInput")
    mask_d = nc.dram_tensor("mask", [128, 896], F16, kind="ExternalInput")
    outT_d = nc.dram_tensor("outT", [D, S], F16, kind="ExternalOutput")

    with tile.TileContext(nc) as tc:
        _emit(nc, tc, (xT_d, wqk_d, qkb_d, wv_d, vb_d, ostk_d, mask_d, outT_d))
    _split_sync_waits(nc)
    return nc


# ---------------------------------------------------------------------------
def _prep_core_inputs(c, x, Qs, Qbs, Ks, Kbs, Vs, Vbs, O):
    b, hg = divmod(c, 4)
    heads = list(range(4 * hg, 4 * hg + 4))
    scale = np.float32(1.0 / np.sqrt(DH))

    xT = np.ascontiguousarray(x[b].T, dtype=np.float16)

    wq = np.concatenate([Qs[h] for h in heads], axis=1)
    wk = np.concatenate([Ks[h] for h in heads], axis=1) * scale
    wqk = np.ascontiguousarray(np.concatenate([wq, wk], axis=1), dtype=np.float16)

    qkb_cols = np.concatenate([Qbs[h] for h in heads] + [Kbs[h] * scale for h in heads])
    qkb = np.ascontiguousarray(qkb_cols.reshape(4, 128).T, dtype=np.float32)

    wv = np.ascontiguousarray(np.concatenate([Vs[h] for h in heads], axis=1),
                              dtype=np.float16)
    vb = np.zeros((128, 260), dtype=np.float16)
    for hh, h in enumerate(heads):
        vb[:, 65 * hh: 65 * hh + 64] = Vbs[h][None, :]

    o4 = np.stack([O[h] for h in heads])                # [4, 64, 1024]
    ostk = np.ascontiguousarray(
        o4.reshape(2, 128, DC, 128).transpose(0, 2, 1, 3), dtype=np.float16)

    t = np.arange(896, dtype=np.int64)[None, :] - 384
    i = np.arange(128, dtype=np.int64)[:, None]
    mask = np.where(t >= i, np.float16(1.0), np.float16(0.0)).astype(np.float16)

    return {"xT": xT, "wqk": wqk, "qkb": qkb, "wv": wv, "vb": vb,
            "ostk": ostk, "mask": np.ascontiguousarray(mask)}


def _run(inputs, trace=False, tmpdir=None):
    x = np.asarray(inputs["normalized_resid_pre"], dtype=np.float32)
    Qs = np.asarray(inputs["Qs"], dtype=np.float32)
    Qbs = np.asarray(inputs["Qbs"], dtype=np.float32)
    Ks = np.asarray(inputs["Ks"], dtype=np.float32)
    Kbs = np.asarray(inputs["Kbs"], dtype=np.float32)
    Vs = np.asarray(inputs["Vs"], dtype=np.float32)
    Vbs = np.asarray(inputs["Vbs"], dtype=np.float32)
    O = np.asarray(inputs["O"], dtype=np.float32)
    Ob = np.asarray(inputs["Ob"], dtype=np.float32)

    nc = build_nc()
    in_maps = [_prep_core_inputs(c, x, Qs, Qbs, Ks, Kbs, Vs, Vbs, O)
               for c in range(8)]
    res = run_bass_kernel_spmd(nc, in_maps, list(range(8)), trace=trace,
                               tmpdir=tmpdir)

    out = np.zeros((B, S, D), dtype=np.float32)
    for c in range(8):
        out[c // 4] += res.results[c]["outT"].T.astype(np.float32)
    out += Ob[None, None, :]
    return out, res


def kernel(**inputs) -> np.ndarray:
    out, _ = _run(inputs, trace=False)
    return out


# revision 30
# speedup vs baseline: 1.0172x; 1.0172x over previous
"""Causal multi-head decoder attention on 8 Trainium2 NeuronCores.

Problem shapes (hardcoded): x [B=2, S=2048, D=1024], 16 heads x d_head=64.
Sharding: core c -> (batch b = c//4, head-group hg = c%4 covering 4 heads).
Attention is fully head-local; each core computes the partial output
projection for its 4 heads, and the host sums the 4 partials per batch
(the "output projection all-reduce") during unshard.

On-device layout strategy (per core):
  - host provides xT = x[b].T  [1024, 2048] so Q/K projections directly
    produce qT/kT [64, S] (head-dim on partitions) with no transposes.
  - V is computed in [S, 64] orientation (x-chunk stationary) and stored
    interleaved with a ones-column per head: vaug [128, 16sc, 4h*65].
    The ones-column makes the attn@V matmul also produce the softmax
    denominator row (zaugT [65, 512] = 64 z rows + 1 denom row).
  - scoresT [ki, qi] = kT-chunk.T @ qT-tile (contraction over d_head=64).
    exp() on the scalar engine (batched over 2 PSUM banks per call);
    causal mask applied additively (-1e5) only on diagonal chunks,
    sliced from one host-built [128, 896] mask tile.
  - 1/sqrt(d_head) folded into the K weights host-side.
  - normalize z by broadcasting reciprocal(denom) across partitions
    (K=1 matmul against a ones row) and multiplying on the vector engine.
  - output projection: O stacked per head-pair so contraction is 128-wide.
"""

import os as _os

import numpy as np

import concourse.bass as bass
import concourse.tile as tile
from concourse import mybir
from concourse.bass_utils import run_bass_kernel_spmd

F32 = mybir.dt.float32
F16 = mybir.dt.float16

B, S, D, NH, DH = 2, 2048, 1024, 16, 64
HL = 4            # heads per core
DC = D // 128     # 8 d-chunks
NQT = S // 512    # 4 qi tiles
NSC = S // 128    # 16 128-token chunks
IGNORE = -100000.0

STAGE = int(_os.environ.get("KERNEL_STAGE", "4"))  # 1=proj 2=+attn 3=+norm 4=full

# ---------------------------------------------------------------------------
# Workaround for this walrus build's per-instruction sync-wait budget of one
# ("Too many sync wait commands"): after Tile scheduling, move excess waits
# from any instruction onto same-engine NoOps inserted just before it.
MAX_WAITS = 1


def _split_sync_waits(nc, max_waits=MAX_WAITS):
    k = 0
    for fn in nc.m.functions:
        for bb in fn.blocks:
            insts = bb.instructions
            i = 0
            while i < len(insts):
                ins = insts[i]
                si = ins.sync_info
                if si is not None and len(si.on_wait) > max_waits:
                    waits = list(si.on_wait)
                    extra, keep = waits[:-max_waits], waits[-max_waits:]
                    for j in range(0, len(extra), max_waits):
                        nop = mybir.InstNoOp(
                            name=nc.get_next_instruction_name(), ins=[], outs=[])
                        k += 1
                        nop.engine = ins.engine
                        nop.sync_info = mybir.SyncInfo(
                            on_wait=extra[j:j + max_waits], on_update=[])
                        nc.register_instruction(nop, overwrite=True)
                        insts.insert(i, nop)
                        i += 1
                    ins.sync_info = mybir.SyncInfo(
                        on_wait=keep, on_update=list(si.on_update))
                i += 1
    return k


# ---------------------------------------------------------------------------
def _emit(nc, tc, d):
    xT_d, wqk_d, qkb_d, wv_d, vb_d, ostk_d, mask_d, outT_d = d

    with tc.tile_pool(name="persist", bufs=1) as persist:
        xT = persist.tile([128, DC, S], F16)
        wqk = persist.tile([128, DC, 512], F16)
        wv = persist.tile([128, DC, 256], F16)
        qkb = persist.tile([128, 4], F32)
        vb = persist.tile([128, 260], F16)
        ostk = persist.tile([128, 2, DC, 128], F16)
        maskt = persist.tile([128, 896], F16)
        qkT = persist.tile([128, 4, S], F16)
        vaug = persist.tile([128, NSC, HL * 65], F16)
        zstk = persist.tile([128, 2, S], F16)
        qdup = persist.tile([128, HL, S], F16)
        kdup = persist.tile([128, HL, S], F16)
        ones64 = persist.tile([128, 64], F32)
        nc.vector.memset(ones64[:, :], 1.0)

        nc.sync.dma_start(out=wqk[:, :, :], in_=wqk_d.rearrange("(c p) n -> p c n", p=128))
        nc.sync.dma_start(out=wv[:, :, :], in_=wv_d.rearrange("(c p) n -> p c n", p=128))
        nc.sync.dma_start(out=qkb[:, :], in_=qkb_d[:, :])
        nc.sync.dma_start(out=vb[:, :], in_=vb_d[:, :])
        nc.sync.dma_start(out=maskt[:, :], in_=mask_d[:, :])
        for dc in range(DC):
            for hh in range(2):
                eng = nc.sync if (dc + hh) % 2 == 0 else nc.scalar
                eng.dma_start(
                    out=xT[:, dc, hh * 1024:(hh + 1) * 1024],
                    in_=xT_d[dc * 128:(dc + 1) * 128, hh * 1024:(hh + 1) * 1024])
        nc.sync.dma_start(out=ostk[:, :, :, :], in_=ostk_d.rearrange("p d r c -> r p d c"))

        with (
            tc.tile_pool(name="psP", bufs=2, space="PSUM") as psP,
            tc.tile_pool(name="psS", bufs=2, space="PSUM") as psS,
            tc.tile_pool(name="psZ", bufs=2, space="PSUM") as psZ,
            tc.tile_pool(name="att", bufs=8) as attp,
            tc.tile_pool(name="nrm", bufs=3) as nrm,
            tc.tile_pool(name="ost", bufs=2) as ostp,
        ):
            for st in range(NQT):
                stw = slice(st * 512, (st + 1) * 512)
                # ---- Q/K projections for this s-tile ----
                for r in range(4):
                    ps = psP.tile([128, 512], F32, tag="proj", name=f"qk_{st}_{r}")
                    for dc in range(DC):
                        nc.tensor.matmul(
                            ps,
                            lhsT=wqk[:, dc, r * 128:(r + 1) * 128],
                            rhs=xT[:, dc, stw],
                            start=(dc == 0), stop=(dc == DC - 1),
                        )
                    nc.vector.tensor_scalar_add(
                        out=qkT[:, r, stw], in0=ps, scalar1=qkb[:, r:r + 1])
                # ---- V projections for this s-tile's four 128-chunks ----
                for sc in range(4 * st, 4 * st + 4):
                    ps = psP.tile([128, 256], F32, tag="proj", name=f"v_{sc}")
                    for dc in range(DC):
                        nc.tensor.matmul(
                            ps,
                            lhsT=xT[:, dc, sc * 128:(sc + 1) * 128],
                            rhs=wv[:, dc, :],
                            start=(dc == 0), stop=(dc == DC - 1),
                        )
                    vsl = vaug[:, sc, :].rearrange("p (h c) -> p h c", c=65)
                    nc.vector.tensor_copy(vsl[:, :, 0:64],
                                          ps.rearrange("p (h c) -> p h c", c=64))
                    nc.vector.memset(vsl[:, :, 64:65], 1.0)
                    nc.vector.tensor_add(out=vaug[:, sc, :], in0=vaug[:, sc, :], in1=vb)
                # ---- duplicate the new qT/kT slices across partition halves
                # so a ki-chunk pair runs as two concurrent row-group matmuls
                for h in range(HL):
                    qrt, rt, pb = h // 2, 2 + h // 2, (h % 2) * 64
                    nc.sync.dma_start(out=qdup[0:64, h, stw], in_=qkT[pb:pb + 64, qrt, stw])
                    nc.sync.dma_start(out=qdup[64:128, h, stw], in_=qkT[pb:pb + 64, qrt, stw])
                    nc.sync.dma_start(out=kdup[0:64, h, stw], in_=qkT[pb:pb + 64, rt, stw])
                    nc.sync.dma_start(out=kdup[64:128, h, stw], in_=qkT[pb:pb + 64, rt, stw])

                # ---- attention for qi-tile st (2 heads in flight) ----
                qt = st
                nkc = 4 * (qt + 1)
                for hp in range(2):
                    heads2 = (2 * hp, 2 * hp + 1)
                    zaugs = {}
                    for h in heads2:
                        zaugs[h] = psZ.tile([65, 512], F32, tag="zaug", name=f"zaug_{h}")
                    for ip in range(nkc // 2):
                        ats = {}
                        for h in heads2:
                            sc2 = psS.tile([128, 1024], F32, tag="sc")
                            for half in range(2):
                                kc = 2 * ip + half
                                lo = half * 64
                                nc.tensor.matmul(
                                    sc2[:, half * 512:(half + 1) * 512],
                                    lhsT=kdup[lo:lo + 64, h, kc * 128:(kc + 1) * 128],
                                    rhs=qdup[lo:lo + 64, h, qt * 512:(qt + 1) * 512],
                                    start=True, stop=True,
                                    tile_position=(lo, 0),
                                )
                            at = attp.tile([128, 1024], F16, tag="at")
                            nc.scalar.activation(out=at, in_=sc2,
                                                 func=mybir.ActivationFunctionType.Exp)
                            for half in range(2):
                                kc = 2 * ip + half
                                j = kc - 4 * qt
                                if 0 <= j < 4:  # diagonal chunk: causal mask
                                    nc.vector.tensor_mul(
                                        out=at[:, half * 512:(half + 1) * 512],
                                        in0=at[:, half * 512:(half + 1) * 512],
                                        in1=maskt[:, 384 - 128 * j: 896 - 128 * j],
                                    )
                            ats[h] = at
                        for h in heads2:
                            for half in range(2):
                                kc = 2 * ip + half
                                nc.tensor.matmul(
                                    zaugs[h],
                                    lhsT=vaug[:, kc, 65 * h: 65 * h + 65],
                                    rhs=ats[h][:, half * 512:(half + 1) * 512],
                                    start=(kc == 0), stop=(kc == nkc - 1),
                                )
                    for h in heads2:
                        zaug = zaugs[h]
                        # normalize: z * exp(-ln(denom)); denom broadcast across
                        # partitions via a K=1 matmul against a ones row.
                        rd = nrm.tile([128, 1024], F32, tag="rd")
                        nc.scalar.activation(out=rd[64:65, 0:512], in_=zaug[64:65, :],
                                             func=mybir.ActivationFunctionType.Ln)
                        nc.scalar.activation(out=rd[64:65, 512:1024],
                                             in_=rd[64:65, 0:512],
                                             func=mybir.ActivationFunctionType.Exp,
                                             scale=-1.0)
                        rb = psS.tile([64, 512], F32, tag="sc")
                        nc.tensor.matmul(rb, lhsT=ones64[64:65, :],
                                         rhs=rd[64:65, 512:1024],
                                         start=True, stop=True)
                        rdb = nrm.tile([64, 512], F32, tag="rdb")
                        nc.vector.tensor_copy(rdb[:, :], rb)
                        pair = h // 2
                        if h % 2 == 0:
                            nc.vector.tensor_mul(out=zstk[0:64, pair, stw],
                                                 in0=zaug[0:64, :], in1=rdb[:, :])
                        else:
                            zs = nrm.tile([64, 512], F16, tag="zs")
                            nc.vector.tensor_mul(out=zs[:, :], in0=zaug[0:64, :],
                                                 in1=rdb[:, :])
                            nc.sync.dma_start(out=zstk[64:128, pair, stw], in_=zs[:, :])
                # ---- output projection for this s-tile ----
                for dc in range(DC):
                    po = psZ.tile([128, 512], F32, tag="zaug", name=f"po_{st}_{dc}")
                    for pair in range(2):
                        nc.tensor.matmul(
                            po,
                            lhsT=ostk[:, pair, dc, :],
                            rhs=zstk[:, pair, stw],
                            start=(pair == 0), stop=(pair == 1),
                        )
                    og = ostp.tile([128, 512], F16, tag="og")
                    nc.vector.tensor_copy(og[:, :], po)
                    nc.sync.dma_start(
                        out=outT_d[dc * 128:(dc + 1) * 128, stw],
                        in_=og[:, :],
                    )


def build_nc() -> bass.Bass:
    nc = bass.Bass()
    xT_d = nc.dram_tensor("xT", [D, S], F16, kind="ExternalInput")
    wqk_d = nc.dram_tensor("wqk", [D, 512], F16, kind="ExternalInput")
    qkb_d = nc.dram_tensor("qkb", [128, 4], F32, kind="ExternalInput")
    wv_d = nc.dram_tensor("wv", [D, 256], F16, kind="ExternalInput")
    vb_d = nc.dram_tensor("vb", [128, 260], F16, kind="ExternalInput")
    ostk_d = nc.dram_tensor("ostk", [2, DC, 128, 128], F16, kind="ExternalInput")
    mask_d = nc.dram_tensor("mask", [128, 896], F16, kind="ExternalInput")
    outT_d = nc.dram_tensor("outT", [D, S], F16, kind="ExternalOutput")

    with tile.TileContext(nc) as tc:
        _emit(nc, tc, (xT_d, wqk_d, qkb_d, wv_d, vb_d, ostk_d, mask_d, outT_d))
    _split_sync_waits(nc)
    return nc


# ---------------------------------------------------------------------------
def _prep_core_inputs(c, x, Qs, Qbs, Ks, Kbs, Vs, Vbs, O):
    b, hg = divmod(c, 4)
    heads = list(range(4 * hg, 4 * hg + 4))
    scale = np.float32(1.0 / np.sqrt(DH))

    xT = np.ascontiguousarray(x[b].T, dtype=np.float16)

    wq = np.concatenate([Qs[h] for h in heads], axis=1)
    wk = np.concatenate([Ks[h] for h in heads], axis=1) * scale
    wqk = np.ascontiguousarray(np.concatenate([wq, wk], axis=1), dtype=np.float16)

    qkb_cols = np.concatenate([Qbs[h] for h in heads] + [Kbs[h] * scale for h in heads])
    qkb = np.ascontiguousarray(qkb_cols.reshape(4, 128).T, dtype=np.float32)

    wv = np.ascontiguousarray(np.concatenate([Vs[h] for h in heads], axis=1),
                              dtype=np.float16)
    vb = np.zeros((128, 260), dtype=np.float16)
    for hh, h in enumerate(heads):
        vb[:, 65 * hh: 65 * hh + 64] = Vbs[h][None, :]

    o4 = np.stack([O[h] for h in heads])                # [4, 64, 1024]
    ostk = np.ascontiguousarray(
        o4.reshape(2, 128, DC, 128).transpose(0, 2, 1, 3), dtype=np.float16)

    t = np.arange(896, dtype=np.int64)[None, :] - 384
    i = np.arange(128, dtype=np.int64)[:, None]
    mask = np.where(t >= i, np.float16(1.0), np.float16(0.0)).astype(np.float16)

    return {"xT": xT, "wqk": wqk, "qkb": qkb, "wv": wv, "vb": vb,
            "ostk": ostk, "mask": np.ascontiguousarray(mask)}


def _run(inputs, trace=False, tmpdir=None):
    x = np.asarray(inputs["normalized_resid_pre"], dtype=np.float32)
    Qs = np.asarray(inputs["Qs"], dtype=np.float32)
    Qbs = np.asarray(inputs["Qbs"], dtype=np.float32)
    Ks = np.asarray(inputs["Ks"], dtype=np.float32)
    Kbs = np.asarray(inputs["Kbs"], dtype=np.float32)
    Vs = np.asarray(inputs["Vs"], dtype=np.float32)
    Vbs = np.asarray(inputs["Vbs"], dtype=np.float32)
    O = np.asarray(inputs["O"], dtype=np.float32)
    Ob = np.asarray(inputs["Ob"], dtype=np.float32)

    nc = build_nc()
    in_maps = [_prep_core_inputs(c, x, Qs, Qbs, Ks, Kbs, Vs, Vbs, O)
               for c in range(8)]
    res = run_bass_kernel_spmd(nc, in_maps, list(range(8)), trace=trace,
                               tmpdir=tmpdir)

    out = np.zeros((B, S, D), dtype=np.float32)
    for c in range(8):
        out[c // 4] += res.results[c]["outT"].T.astype(np.float32)
    out += Ob[None, None, :]
    return out, res


def kernel(**inputs) -> np.ndarray:
    out, _ = _run(inputs, trace=False)
    return out
